# revision 7
# baseline (speedup 1.0000x reference)
"""BitLinear (ternary weight / int8-activation quantized matmul) Trainium2 kernel.

Reference semantics (for x:(B,S,D), weight:(O,D)):
    alpha = max(mean(|W|), 1e-8)                     # per-tensor scalar
    w_q   = clip(round(W/alpha), -1, 1)              # ternary
    beta  = max(max|x| / 127, 1e-8)                  # per token
    x_q   = clip(round(x/beta), -127, 127)           # int8 range
    y     = (x_q @ w_q.T) * alpha * beta

Sharding: data-parallel over the 16384 tokens across 8 NeuronCores
(2048 tokens/core); full weight replicated per core (no collectives).
The quantized GEMM runs in bf16 which is EXACT here: x_q in [-127,127]
and w_q in {-1,0,1} are exactly representable in bf16 and all partial
sums stay far below 2^24, so fp32 PSUM accumulation is exact.

Rounding uses the fp32 magic-number trick ((v + 1.5*2^23) - 1.5*2^23)
which implements round-half-to-even, matching jnp.round bit-for-bit.
"""

import numpy as np

import bass_rust
import concourse.bass as bass
import concourse.mybir as mybir
import concourse.tile as tile
from concourse.bass_utils import run_bass_kernel_spmd
from concourse.masks import make_identity

N_CORES = 8
P = 128
MAGIC = 12582912.0  # 1.5 * 2**23 : fp32 RNE round-to-integer magic constant
EPS = 1e-8

# Full-problem shapes (hardcoded per the grading contract)
FULL_B, FULL_S, FULL_D = 4, 4096, 2048
D_IN = 2048
D_OUT = 2048
TOK_PER_CORE = FULL_B * FULL_S // N_CORES  # 2048


def _split_excess_waits(nc, max_waits=1):
    """This container's walrus accepts at most `max_waits` sync waits per
    instruction; move excess waits onto preceding same-engine nops."""
    n = 0
    for f in nc.m.functions:
        for bb in f.blocks:
            insts = list(bb.instructions)
            out = []
            changed = False
            for inst in insts:
                si = inst.sync_info
                if si is not None and len(si.on_wait) > max_waits:
                    waits = list(si.on_wait)
                    extra, keep = waits[:-max_waits], waits[-max_waits:]
                    for i in range(0, len(extra), max_waits):
                        chunk = extra[i : i + max_waits]
                        n += 1
                        nop = mybir.InstNoOp(name=f"waitsplit-{n}")
                        nop.engine = inst.engine
                        nop.sync_info = bass_rust.SyncInfo(
                            on_wait=chunk, on_update=[]
                        )
                        out.append(nop)
                    inst.sync_info = bass_rust.SyncInfo(
                        on_wait=keep, on_update=list(si.on_update)
                    )
                    changed = True
                out.append(inst)
            if changed:
                bb.instructions = out


def emit_bitlinear(tc, y_ap, x_ap, wt_ap, d_in, d_out, n_tok):
    """Emit the per-core kernel body.

    x_ap:  [n_tok, d_in]  f32 token rows for this core
    wt_ap: [d_in, d_out]  f32 transposed weight (wt[i,o] = W[o,i])
    y_ap:  [n_tok, d_out] f32 output
    """
    from contextlib import ExitStack

    nc = tc.nc
    f32 = mybir.dt.float32
    bf16 = mybir.dt.bfloat16
    NK = d_in // P
    NO = d_out // 512
    NX = n_tok // P
    inv_n = 1.0 / float(d_in * d_out)  # power of two for our shapes => exact

    with ExitStack() as ctx:
        const = ctx.enter_context(tc.tile_pool(name="const", bufs=1))
        wf32 = ctx.enter_context(tc.tile_pool(name="wf32", bufs=3))
        small = ctx.enter_context(tc.tile_pool(name="small", bufs=4))
        qtmp = ctx.enter_context(tc.tile_pool(name="qtmp", bufs=2))
        wqtp = ctx.enter_context(tc.tile_pool(name="wqtp", bufs=1))
        xf32 = ctx.enter_context(tc.tile_pool(name="xf32", bufs=3))
        xqp = ctx.enter_context(tc.tile_pool(name="xqp", bufs=2))
        xqtp = ctx.enter_context(tc.tile_pool(name="xqtp", bufs=2))
        yout = ctx.enter_context(tc.tile_pool(name="yout", bufs=3))
        pyp = ctx.enter_context(tc.tile_pool(name="pyp", bufs=1, space="PSUM"))
        ptp = ctx.enter_context(tc.tile_pool(name="ptp", bufs=3, space="PSUM"))
        pap = ctx.enter_context(tc.tile_pool(name="pap", bufs=1, space="PSUM"))

        ident = const.tile([P, P], bf16)
        make_identity(nc, ident)
        ones_k = const.tile([P, 1], f32)
        nc.vector.memset(ones_k, 1.0)
        ones_m = const.tile([1, P], f32)
        nc.vector.memset(ones_m, 1.0)

        # ---- Phase 1: alpha = max(mean|W|, EPS), inv_alpha = 1/alpha ----
        # |W| row-sums on the (otherwise idle) Scalar engine via
        # activation(Abs, accum_out) so the DVE is free for quantization.
        partials = const.tile([P, NK], f32)
        for j in range(NK):
            wj = wf32.tile([P, d_out], f32, tag="wj")
            nc.sync.dma_start(out=wj, in_=wt_ap[j * P : (j + 1) * P, :])
            trash = qtmp.tile([P, d_out], bf16, tag="trash")
            nc.scalar.activation(
                out=trash,
                in_=wj,
                func=mybir.ActivationFunctionType.Abs,
                accum_out=partials[:, j : j + 1],
            )
        total = const.tile([P, 1], f32)
        nc.vector.tensor_reduce(
            out=total,
            in_=partials,
            axis=mybir.AxisListType.X,
            op=mybir.AluOpType.add,
        )
        pa_sum = pap.tile([1, 1], f32, tag="pa")
        nc.tensor.matmul(pa_sum, lhsT=total, rhs=ones_k, start=True, stop=True)
        scal = const.tile([1, 2], f32)
        nc.vector.tensor_scalar(
            scal[:, 0:1], pa_sum, inv_n, EPS,
            mybir.AluOpType.mult, mybir.AluOpType.max,
        )
        nc.vector.reciprocal(out=scal[:, 1:2], in_=scal[:, 0:1])
        pa_bc = pap.tile([P, 2], f32, tag="pa")
        nc.tensor.matmul(pa_bc, lhsT=ones_m, rhs=scal, start=True, stop=True)
        ab = const.tile([P, 2], f32)
        nc.scalar.copy(out=ab, in_=pa_bc)
        alpha_bc = ab[:, 0:1]
        invalpha_bc = ab[:, 1:2]

        # ---- Phase 2: W_qT = clip(round(wT * inv_alpha), -1, 1) in bf16 ----
        # Per-k tiles so x-tile matmuls can start as soon as slice k is ready.
        wqt_tiles = []
        for j in range(NK):
            wj = wf32.tile([P, d_out], f32, tag="wj")
            nc.sync.dma_start(out=wj, in_=wt_ap[j * P : (j + 1) * P, :])
            q = qtmp.tile([P, d_out], f32, tag="q32")
            nc.vector.tensor_scalar(
                q, wj, invalpha_bc, MAGIC,
                mybir.AluOpType.mult, mybir.AluOpType.add,
            )
            r = qtmp.tile([P, d_out], bf16, tag="wr")
            nc.vector.tensor_scalar(
                r, q, MAGIC, -1.0,
                mybir.AluOpType.subtract, mybir.AluOpType.max,
            )
            wq_j = wqtp.tile([P, d_out], bf16, tag=f"wqt{j}")
            nc.gpsimd.tensor_scalar(
                wq_j, r, 1.0, None, mybir.AluOpType.min,
            )
            wqt_tiles.append(wq_j)

        # ---- Phase 3: per 128-token tile: quantize, transpose, matmul ----
        for i in range(NX):
            xi = xf32.tile([P, d_in], f32, tag="xi")
            nc.sync.dma_start(out=xi, in_=x_ap[i * P : (i + 1) * P, :])
            am = small.tile([P, 1], f32, tag="am")
            nc.vector.tensor_reduce(
                out=am,
                in_=xi,
                axis=mybir.AxisListType.X,
                op=mybir.AluOpType.max,
                apply_absolute_value=True,
            )
            beta = small.tile([P, 1], f32, tag="beta")
            nc.vector.tensor_scalar(
                beta, am, 1.0 / 127.0, EPS,
                mybir.AluOpType.mult, mybir.AluOpType.max,
            )
            invb = small.tile([P, 1], f32, tag="invb")
            nc.vector.reciprocal(out=invb, in_=beta)
            scale = small.tile([P, 1], f32, tag="scale")
            nc.vector.tensor_scalar(
                scale, beta, alpha_bc, None, mybir.AluOpType.mult,
            )
            q = qtmp.tile([P, d_in], f32, tag="q32")
            nc.vector.tensor_scalar(
                q, xi, invb, MAGIC,
                mybir.AluOpType.mult, mybir.AluOpType.add,
            )
            xq = xqp.tile([P, d_in], bf16, tag="xq")
            nc.gpsimd.tensor_scalar(
                xq, q, MAGIC, None, mybir.AluOpType.subtract,
            )
            xqt = xqtp.tile([P, NK, P], bf16, tag="xqt")
            for j in range(NK):
                pt = ptp.tile([P, P], bf16, tag="pt")
                nc.tensor.transpose(pt, xq[:, j * P : (j + 1) * P], ident)
                nc.scalar.copy(out=xqt[:, j, :], in_=pt)
            py = pyp.tile([P, d_out], f32, tag="py")
            for k in range(NK):
                for b in range(NO):
                    nc.tensor.matmul(
                        py[:, b * 512 : (b + 1) * 512],
                        lhsT=xqt[:, k, :],
                        rhs=wqt_tiles[k][:, b * 512 : (b + 1) * 512],
                        start=(k == 0),
                        stop=(k == NK - 1),
                    )
            ysb = yout.tile([P, d_out], f32, tag="ysb")
            nc.scalar.mul(out=ysb, in_=py, mul=scale)
            nc.sync.dma_start(out=y_ap[i * P : (i + 1) * P, :], in_=ysb)


def build_nc(d_in=D_IN, d_out=D_OUT, n_tok=TOK_PER_CORE, n_cores=N_CORES):
    nc = bass.Bass(
        "TRN2", target_bir_lowering=False, debug=False, num_devices=n_cores
    )
    x = nc.dram_tensor("x", [n_tok, d_in], mybir.dt.float32, kind="ExternalInput")
    wt = nc.dram_tensor("wt", [d_in, d_out], mybir.dt.float32, kind="ExternalInput")
    y = nc.dram_tensor("y", [n_tok, d_out], mybir.dt.float32, kind="ExternalOutput")
    with tile.TileContext(nc) as tc:
        emit_bitlinear(tc, y[:, :], x[:, :], wt[:, :], d_in, d_out, n_tok)
    _split_excess_waits(nc)
    return nc


_NC_CACHE = {}


def _run(x: np.ndarray, weight: np.ndarray, **spmd_kwargs):
    x = np.ascontiguousarray(np.asarray(x, dtype=np.float32))
    weight = np.asarray(weight, dtype=np.float32)
    b, s, d = x.shape
    n_tok_full = b * s
    n_tok = n_tok_full // N_CORES
    wt = np.ascontiguousarray(weight.T)

    key = (d, weight.shape[0], n_tok)
    if key not in _NC_CACHE:
        _NC_CACHE[key] = build_nc(d_in=d, d_out=weight.shape[0], n_tok=n_tok)
    nc = _NC_CACHE[key]

    x2d = x.reshape(n_tok_full, d)
    in_maps = [
        {"x": x2d[c * n_tok : (c + 1) * n_tok], "wt": wt} for c in range(N_CORES)
    ]
    res = run_bass_kernel_spmd(
        nc, in_maps, core_ids=list(range(N_CORES)), **spmd_kwargs
    )
    y = np.concatenate([res.results[c]["y"] for c in range(N_CORES)], axis=0)
    return y.reshape(b, s, weight.shape[0]), res


def kernel(x: np.ndarray, weight: np.ndarray) -> np.ndarray:
    y, _ = _run(x, weight)
    return y


# revision 9
# speedup vs baseline: 2.6514x; 2.6514x over previous
"""BitLinear (ternary weight / int8-activation quantized matmul) Trainium2 kernel.

Reference semantics (for x:(B,S,D), weight:(O,D)):
    alpha = max(mean(|W|), 1e-8)                     # per-tensor scalar
    w_q   = clip(round(W/alpha), -1, 1)              # ternary
    beta  = max(max|x| / 127, 1e-8)                  # per token
    x_q   = clip(round(x/beta), -127, 127)           # int8 range
    y     = (x_q @ w_q.T) * alpha * beta

Sharding: data-parallel over the 16384 tokens across 8 NeuronCores
(2048 tokens/core); full weight replicated per core (no collectives).
The quantized GEMM runs in bf16 which is EXACT here: x_q in [-127,127]
and w_q in {-1,0,1} are exactly representable in bf16 and all partial
sums stay far below 2^24, so fp32 PSUM accumulation is exact.

Rounding uses the fp32 magic-number trick ((v + 1.5*2^23) - 1.5*2^23)
which implements round-half-to-even, matching jnp.round bit-for-bit.
"""

import numpy as np

import bass_rust
import concourse.bass as bass
import concourse.mybir as mybir
import concourse.tile as tile
from concourse.bass_utils import run_bass_kernel_spmd
from concourse.masks import make_identity

N_CORES = 8
P = 128
MAGIC = 12582912.0  # 1.5 * 2**23 : fp32 RNE round-to-integer magic constant
EPS = 1e-8

# Full-problem shapes (hardcoded per the grading contract)
FULL_B, FULL_S, FULL_D = 4, 4096, 2048
D_IN = 2048
D_OUT = 2048
TOK_PER_CORE = FULL_B * FULL_S // N_CORES  # 2048


def _split_excess_waits(nc, max_waits=1):
    """This container's walrus accepts at most `max_waits` sync waits per
    instruction; move excess waits onto preceding same-engine nops."""
    n = 0
    for f in nc.m.functions:
        for bb in f.blocks:
            insts = list(bb.instructions)
            out = []
            changed = False
            for inst in insts:
                si = inst.sync_info
                if si is not None and len(si.on_wait) > max_waits:
                    waits = list(si.on_wait)
                    extra, keep = waits[:-max_waits], waits[-max_waits:]
                    for i in range(0, len(extra), max_waits):
                        chunk = extra[i : i + max_waits]
                        n += 1
                        nop = mybir.InstNoOp(name=f"waitsplit-{n}")
                        nop.engine = inst.engine
                        nop.sync_info = bass_rust.SyncInfo(
                            on_wait=chunk, on_update=[]
                        )
                        out.append(nop)
                    inst.sync_info = bass_rust.SyncInfo(
                        on_wait=keep, on_update=list(si.on_update)
                    )
                    changed = True
                out.append(inst)
            if changed:
                bb.instructions = out


def emit_bitlinear(tc, y_ap, x_ap, wt_ap, d_in, d_out, n_tok):
    """Emit the per-core kernel body.

    x_ap:  [n_tok, d_in]  f32 token rows for this core
    wt_ap: [d_in, d_out]  f32 transposed weight (wt[i,o] = W[o,i])
    y_ap:  [n_tok, d_out] f32 output
    """
    from contextlib import ExitStack

    nc = tc.nc
    f32 = mybir.dt.float32
    bf16 = mybir.dt.bfloat16
    NK = d_in // P
    NO = d_out // 512
    NX = n_tok // P
    inv_n = 1.0 / float(d_in * d_out)  # power of two for our shapes => exact

    with ExitStack() as ctx:
        const = ctx.enter_context(tc.tile_pool(name="const", bufs=1))
        wf32 = ctx.enter_context(tc.tile_pool(name="wf32", bufs=3))
        small = ctx.enter_context(tc.tile_pool(name="small", bufs=4))
        qtmp = ctx.enter_context(tc.tile_pool(name="qtmp", bufs=2))
        wqtp = ctx.enter_context(tc.tile_pool(name="wqtp", bufs=1))
        xf32 = ctx.enter_context(tc.tile_pool(name="xf32", bufs=3))
        xqp = ctx.enter_context(tc.tile_pool(name="xqp", bufs=2))
        xqtp = ctx.enter_context(tc.tile_pool(name="xqtp", bufs=2))
        yout = ctx.enter_context(tc.tile_pool(name="yout", bufs=3))
        pyp = ctx.enter_context(tc.tile_pool(name="pyp", bufs=1, space="PSUM"))
        ptp = ctx.enter_context(tc.tile_pool(name="ptp", bufs=3, space="PSUM"))
        pap = ctx.enter_context(tc.tile_pool(name="pap", bufs=1, space="PSUM"))

        ident = const.tile([P, P], bf16)
        make_identity(nc, ident)
        ones_k = const.tile([P, 1], f32)
        nc.vector.memset(ones_k, 1.0)
        ones_m = const.tile([1, P], f32)
        nc.vector.memset(ones_m, 1.0)

        # ---- Phase 1: alpha = max(mean|W|, EPS), inv_alpha = 1/alpha ----
        # |W| row-sums on the (otherwise idle) Scalar engine via
        # activation(Abs, accum_out) so the DVE is free for quantization.
        partials = const.tile([P, NK], f32)
        for j in range(NK):
            wj = wf32.tile([P, d_out], f32, tag="wj")
            nc.sync.dma_start(out=wj, in_=wt_ap[j * P : (j + 1) * P, :])
            trash = qtmp.tile([P, d_out], bf16, tag="trash")
            nc.scalar.activation(
                out=trash,
                in_=wj,
                func=mybir.ActivationFunctionType.Abs,
                accum_out=partials[:, j : j + 1],
            )
        total = const.tile([P, 1], f32)
        nc.vector.tensor_reduce(
            out=total,
            in_=partials,
            axis=mybir.AxisListType.X,
            op=mybir.AluOpType.add,
        )
        pa_sum = pap.tile([1, 1], f32, tag="pa")
        nc.tensor.matmul(pa_sum, lhsT=total, rhs=ones_k, start=True, stop=True)
        scal = const.tile([1, 2], f32)
        nc.vector.tensor_scalar(
            scal[:, 0:1], pa_sum, inv_n, EPS,
            mybir.AluOpType.mult, mybir.AluOpType.max,
        )
        nc.vector.reciprocal(out=scal[:, 1:2], in_=scal[:, 0:1])
        pa_bc = pap.tile([P, 2], f32, tag="pa")
        nc.tensor.matmul(pa_bc, lhsT=ones_m, rhs=scal, start=True, stop=True)
        ab = const.tile([P, 2], f32)
        nc.scalar.copy(out=ab, in_=pa_bc)
        alpha_bc = ab[:, 0:1]
        invalpha_bc = ab[:, 1:2]

        # ---- Phase 2: W_qT = clip(round(wT * inv_alpha), -1, 1) in bf16 ----
        # Per-k tiles so x-tile matmuls can start as soon as slice k is ready.
        wqt_tiles = []
        for j in range(NK):
            wj = wf32.tile([P, d_out], f32, tag="wj")
            nc.sync.dma_start(out=wj, in_=wt_ap[j * P : (j + 1) * P, :])
            q = qtmp.tile([P, d_out], f32, tag="q32")
            nc.vector.tensor_scalar(
                q, wj, invalpha_bc, MAGIC,
                mybir.AluOpType.mult, mybir.AluOpType.add,
            )
            r = qtmp.tile([P, d_out], bf16, tag="wr")
            nc.vector.tensor_scalar(
                r, q, MAGIC, -1.0,
                mybir.AluOpType.subtract, mybir.AluOpType.max,
            )
            wq_j = wqtp.tile([P, d_out], bf16, tag=f"wqt{j}")
            nc.vector.tensor_scalar(
                wq_j, r, 1.0, None, mybir.AluOpType.min,
            )
            wqt_tiles.append(wq_j)

        # ---- Phase 3: per 128-token tile: quantize, transpose, matmul ----
        for i in range(NX):
            xi = xf32.tile([P, d_in], f32, tag="xi")
            nc.sync.dma_start(out=xi, in_=x_ap[i * P : (i + 1) * P, :])
            am = small.tile([P, 1], f32, tag="am")
            nc.vector.tensor_reduce(
                out=am,
                in_=xi,
                axis=mybir.AxisListType.X,
                op=mybir.AluOpType.max,
                apply_absolute_value=True,
            )
            beta = small.tile([P, 1], f32, tag="beta")
            nc.vector.tensor_scalar(
                beta, am, 1.0 / 127.0, EPS,
                mybir.AluOpType.mult, mybir.AluOpType.max,
            )
            invb = small.tile([P, 1], f32, tag="invb")
            nc.vector.reciprocal(out=invb, in_=beta)
            scale = small.tile([P, 1], f32, tag="scale")
            nc.vector.tensor_scalar(
                scale, beta, alpha_bc, None, mybir.AluOpType.mult,
            )
            q = qtmp.tile([P, d_in], f32, tag="q32")
            nc.vector.tensor_scalar(
                q, xi, invb, MAGIC,
                mybir.AluOpType.mult, mybir.AluOpType.add,
            )
            xq = xqp.tile([P, d_in], bf16, tag="xq")
            nc.vector.tensor_scalar(
                xq, q, MAGIC, None, mybir.AluOpType.subtract,
            )
            xqt = xqtp.tile([P, NK, P], bf16, tag="xqt")
            for j in range(NK):
                pt = ptp.tile([P, P], bf16, tag="pt")
                nc.tensor.transpose(pt, xq[:, j * P : (j + 1) * P], ident)
                nc.scalar.copy(out=xqt[:, j, :], in_=pt)
            py = pyp.tile([P, d_out], f32, tag="py")
            for k in range(NK):
                for b in range(NO):
                    nc.tensor.matmul(
                        py[:, b * 512 : (b + 1) * 512],
                        lhsT=xqt[:, k, :],
                        rhs=wqt_tiles[k][:, b * 512 : (b + 1) * 512],
                        start=(k == 0),
                        stop=(k == NK - 1),
                    )
            ysb = yout.tile([P, d_out], f32, tag="ysb")
            nc.scalar.mul(out=ysb, in_=py, mul=scale)
            nc.sync.dma_start(out=y_ap[i * P : (i + 1) * P, :], in_=ysb)


def build_nc(d_in=D_IN, d_out=D_OUT, n_tok=TOK_PER_CORE, n_cores=N_CORES):
    nc = bass.Bass(
        "TRN2", target_bir_lowering=False, debug=False, num_devices=n_cores
    )
    x = nc.dram_tensor("x", [n_tok, d_in], mybir.dt.float32, kind="ExternalInput")
    wt = nc.dram_tensor("wt", [d_in, d_out], mybir.dt.float32, kind="ExternalInput")
    y = nc.dram_tensor("y", [n_tok, d_out], mybir.dt.float32, kind="ExternalOutput")
    with tile.TileContext(nc) as tc:
        emit_bitlinear(tc, y[:, :], x[:, :], wt[:, :], d_in, d_out, n_tok)
    _split_excess_waits(nc)
    return nc


_NC_CACHE = {}


def _run(x: np.ndarray, weight: np.ndarray, **spmd_kwargs):
    x = np.ascontiguousarray(np.asarray(x, dtype=np.float32))
    weight = np.asarray(weight, dtype=np.float32)
    b, s, d = x.shape
    n_tok_full = b * s
    n_tok = n_tok_full // N_CORES
    wt = np.ascontiguousarray(weight.T)

    key = (d, weight.shape[0], n_tok)
    if key not in _NC_CACHE:
        _NC_CACHE[key] = build_nc(d_in=d, d_out=weight.shape[0], n_tok=n_tok)
    nc = _NC_CACHE[key]

    x2d = x.reshape(n_tok_full, d)
    in_maps = [
        {"x": x2d[c * n_tok : (c + 1) * n_tok], "wt": wt} for c in range(N_CORES)
    ]
    res = run_bass_kernel_spmd(
        nc, in_maps, core_ids=list(range(N_CORES)), **spmd_kwargs
    )
    y = np.concatenate([res.results[c]["y"] for c in range(N_CORES)], axis=0)
    return y.reshape(b, s, weight.shape[0]), res


def kernel(x: np.ndarray, weight: np.ndarray) -> np.ndarray:
    y, _ = _run(x, weight)
    return y
